# revision 77
# baseline (speedup 1.0000x reference)
"""GroupedQueryAttention TRN2 kernel (v5).

Full inputs in, full output out. Sharding: 8 cores = (batch 2) x (head
quarter 4). Each core computes attention for 8 heads / 2 KV groups of one
batch element and a row-sharded out-projection partial; the host sums the
4 partials per batch element.

Data path is bf16 on SBUF (PSUM accumulation stays fp32):
 - RoPE rotate-half is a signed-permutation matmul on the tensor engine
   (rot(x) * sin == Perm @ (x * sin), valid because the rope tables'
   half-rows are identical), keeping the rope chain on DVE+PE only.
 - V^T is produced directly by swapping matmul operands (x slab is the
   stationary operand); all 16 V^T units are deferred into the attention
   phase where they fill tensor-engine gaps left by exp latency in the
   early (small) query blocks.
 - Causal masking of the diagonal 128x128 stripes runs on the Pool
   engine (affine_select); softmax denominators broadcast via gpsimd
   partition_broadcast; the ones-column on V yields the denominator
   inside the ctx matmul.
 - Out-projection of query block qb-1 is emit-interleaved into the head
   loop of qb; the last block's out-projection uses ACT-engine copies in
   the tail where ACT is idle.
"""

import math

import numpy as np

# Full-problem constants (hardcoded per contract).
B, S, D = 2, 2048, 2048
H, G, HD = 32, 8, 64
N_CORES = 8
TP = 4  # head-parallel ways per batch element


def _cfg_full():
    return dict(
        S=S,
        D=D,
        HL=H // TP,      # 8 local heads
        GL=G // TP,      # 2 local kv groups
        HD=HD,
        TB=512,          # token block for the projection phase
        QB=512,          # query block for attention
    )


def build_core_program(cfg, num_devices=8):
    """Emit the per-core SPMD program. Returns the compiled Bass object."""
    import concourse.bacc as bacc
    import concourse.mybir as mybir
    from concourse import tile

    f32 = mybir.dt.float32
    bf16 = mybir.dt.bfloat16

    Sc, Dc, HL, GL, HDc, TB, QB = (
        cfg["S"], cfg["D"], cfg["HL"], cfg["GL"], cfg["HD"], cfg["TB"], cfg["QB"]
    )
    KC = Dc // 128                    # contraction chunks over D
    CH = HL * HDc + 2 * GL * HDc      # fused qkv channels (q | k | v)
    QMT = HL * HDc // 128             # q M-tiles (2 heads each)
    SC = Sc // 128                    # key chunks of 128
    CC = HL * HDc // 128              # ctx channel chunks
    NTB = Sc // TB
    NQB = Sc // QB
    GS = HL // GL                     # heads per local group
    DOUT = Dc
    KOF = HL * HDc                    # k column offset in wqkv
    VOF = KOF + GL * HDc              # v column offset
    scale = 1.0 / math.sqrt(HDc)

    nc = bacc.Bacc(
        "TRN2",
        target_bir_lowering=False,
        debug=False,
        enable_asserts=False,
        num_devices=num_devices,
    )

    xt_d = nc.dram_tensor("xT", [Dc, Sc], bf16, kind="ExternalInput").ap()
    xt_v = xt_d.rearrange("(k p) t -> p k t", p=128)
    wqkv_d = nc.dram_tensor("wqkv", [Dc, CH], bf16, kind="ExternalInput").ap()
    wo_d = nc.dram_tensor("wo", [HL * HDc, DOUT], bf16, kind="ExternalInput").ap()
    cos_d = nc.dram_tensor("cosT", [128, Sc], f32, kind="ExternalInput").ap()
    sin_d = nc.dram_tensor("sinT", [128, Sc], f32, kind="ExternalInput").ap()
    perm_d = nc.dram_tensor("permT", [128, 128], bf16, kind="ExternalInput").ap()
    out_d = nc.dram_tensor("out", [Sc, DOUT], bf16,
                           kind="ExternalOutput").ap()

    with tile.TileContext(nc) as tc:
        with tc.tile_pool(name="persist", bufs=1) as pp:
            cosT = pp.tile([128, Sc], f32)
            sinT = pp.tile([128, Sc], f32)
            permT = pp.tile([128, 128], bf16)
            QT = pp.tile([128, QMT, Sc], bf16)   # [2-head rows, pair, tokens]
            KT2 = pp.tile([128, Sc], bf16)       # g0 rows 0:64, g1 rows 64:128
            KTx = pp.tile([128, Sc], bf16)       # swapped halves of KT2
            Vplus = [pp.tile([128, SC, HDc + 1], bf16, tag=f"vplus{g}",
                             name=f"vplus{g}") for g in range(GL)]
            wqkv_sb = pp.tile([128, KC, CH], bf16)

            for g in range(GL):
                nc.vector.memset(Vplus[g][:, :, HDc], 1.0)
            dmask = pp.tile([128, 128], bf16)
            nc.gpsimd.memset(dmask[:], 1.0)
            nc.gpsimd.affine_select(
                out=dmask[:], in_=dmask[:],
                compare_op=mybir.AluOpType.is_ge,
                fill=0.0, base=0, pattern=[[1, 128]], channel_multiplier=-1,
            )

            with tc.tile_pool(name="slab", bufs=4) as slabp:
                slabs = {}
                slabs[0] = slabp.tile([128, KC, TB], bf16, tag="slab",
                                      name="slab0")
                # interleave weight + first-slab loads so the kc=0 matmul
                # can start after ~2 transfers instead of all 32
                wqkv_v = wqkv_d.rearrange("(k p) c -> p k c", p=128)
                for kc in range(0, KC, 2):
                    nc.sync.dma_start(
                        out=wqkv_sb[:, kc:kc + 2, :],
                        in_=wqkv_v[:, kc:kc + 2, :],
                    )
                    nc.sync.dma_start(
                        out=slabs[0][:, kc:kc + 2, :],
                        in_=xt_v[:, kc:kc + 2, 0:TB],
                    )
                    if kc == 4:
                        nc.sync.dma_start(out=sinT[:, 0:512],
                                          in_=sin_d[:, 0:512])
                        nc.sync.dma_start(out=cosT[:, 0:512],
                                          in_=cos_d[:, 0:512])
                        nc.sync.dma_start(out=permT[:], in_=perm_d[:])


                # ---------------- phase 1: q/k projection + rope --------
                with (
                    tc.tile_pool(name="scr", bufs=3) as scrp,
                    tc.tile_pool(name="pj", bufs=4, space="PSUM") as pjp,
                    tc.tile_pool(name="rot", bufs=2, space="PSUM") as rotp,
                ):
                    def rope_unit(dst, ps, ts, un):
                        # dst = ps*cos + Perm @ (ps*sin)
                        sins = scrp.tile([128, TB], bf16, tag="sins",
                                         name=f"sins{un}")
                        nc.vector.tensor_mul(sins[:], ps[:], sinT[:, ts])
                        rot = rotp.tile([128, TB], f32, tag="rot",
                                        name=f"rot{un}")
                        nc.tensor.matmul(rot[:], permT[:], sins[:],
                                         start=True, stop=True)
                        qcos = scrp.tile([128, TB], f32, tag="qcos",
                                         name=f"qcos{un}")
                        nc.vector.tensor_mul(qcos[:], ps[:], cosT[:, ts])
                        nc.vector.tensor_add(dst, qcos[:], rot[:])

                    for tb in range(NTB):
                        ts = slice(tb * TB, (tb + 1) * TB)
                        if tb > 0:
                            slabs[tb] = slabp.tile([128, KC, TB], bf16,
                                                   tag="slab",
                                                   name=f"slab{tb}")
                            nc.sync.dma_start(out=slabs[tb][:],
                                              in_=xt_v[:, :, ts])
                            nc.sync.dma_start(out=sinT[:, ts],
                                              in_=sin_d[:, ts])
                            nc.sync.dma_start(out=cosT[:, ts],
                                              in_=cos_d[:, ts])
                        slab = slabs[tb]
                        for mt in range(QMT + 1):
                            col = mt * 128 if mt < QMT else KOF
                            ps = pjp.tile([128, TB], f32, tag="pj")
                            for kc in range(KC):
                                nc.tensor.matmul(
                                    ps[:],
                                    wqkv_sb[:, kc, col:col + 128],
                                    slab[:, kc, :],
                                    start=(kc == 0), stop=(kc == KC - 1),
                                )
                            if mt < QMT:
                                rope_unit(QT[:, mt, ts], ps, ts,
                                          f"{tb}_{mt}")
                            else:
                                rope_unit(KT2[:, ts], ps, ts, f"{tb}_k")
                                nc.sync.dma_start(out=KTx[0:64, ts],
                                                  in_=KT2[64:128, ts])
                                nc.sync.dma_start(out=KTx[64:128, ts],
                                                  in_=KT2[0:64, ts])

                # ---------------- phase 2: V^T + attention + outproj ----
                with (
                    tc.tile_pool(name="p2", bufs=1) as p2,
                    tc.tile_pool(name="expp", bufs=4) as expp,
                    tc.tile_pool(name="dens", bufs=3) as densp,
                    tc.tile_pool(name="ctxp", bufs=3) as ctxp,
                    tc.tile_pool(name="outp", bufs=4) as outp,
                    tc.tile_pool(name="sc_ps", bufs=2, space="PSUM") as scps,
                    tc.tile_pool(name="ctx_ps", bufs=2, space="PSUM") as ctxps,
                    tc.tile_pool(name="mm_ps", bufs=2, space="PSUM") as mmps,
                ):
                    wo_sb = p2.tile([128, CC, DOUT], bf16)
                    nc.sync.dma_start(
                        out=wo_sb[:],
                        in_=wo_d.rearrange("(c p) d -> p c d", p=128),
                    )

                    def vt_unit(sc_i):
                        """Direct V^T for one 128-token chunk."""
                        tb, tcc = divmod(sc_i, TB // 128)
                        slab = slabs[tb]
                        vt = mmps.tile([128, 128], f32, tag="mm",
                                       name=f"vt{sc_i}")
                        for kc in range(KC):
                            nc.tensor.matmul(
                                vt[:],
                                slab[:, kc, tcc * 128:(tcc + 1) * 128],
                                wqkv_sb[:, kc, VOF:VOF + 128],
                                start=(kc == 0), stop=(kc == KC - 1),
                            )
                        for g in range(GL):
                            nc.vector.tensor_copy(
                                Vplus[g][:, sc_i, 0:HDc],
                                vt[:, g * HDc:(g + 1) * HDc],
                            )

                    def kslice(g, rb, kc):
                        ks = slice(kc * 128, (kc + 1) * 128)
                        if g == 0:
                            return KT2[0:64, ks] if rb == 0 else KTx[64:128, ks]
                        return KT2[64:128, ks] if rb == 64 else KTx[0:64, ks]

                    osb_cur = {}

                    def outproj_block(ctx_sb, qb, tt, db, tail=False):
                        po = mmps.tile([128, 512], f32, tag="mm",
                                       name=f"po{qb}_{tt}_{db}")
                        for cc in range(CC):
                            nc.tensor.matmul(
                                po[:],
                                ctx_sb[:, cc, tt * 128:(tt + 1) * 128],
                                wo_sb[:, cc, db * 512:(db + 1) * 512],
                                start=(cc == 0), stop=(cc == CC - 1),
                            )
                        key = (qb, tt)
                        if key not in osb_cur:
                            osb_cur[key] = [outp.tile([128, DOUT], bf16,
                                                      tag="out",
                                                      name=f"osb{qb}_{tt}"),
                                            0]
                        osb, done = osb_cur[key]
                        if tail == "act":
                            nc.scalar.copy(osb[:, db * 512:(db + 1) * 512],
                                           po[:])
                        else:
                            nc.vector.tensor_copy(
                                osb[:, db * 512:(db + 1) * 512], po[:])
                        osb_cur[key][1] = done + 1
                        t0 = qb * QB + tt * 128
                        if tail:
                            # fine-grained drain at the kernel end
                            nc.sync.dma_start(
                                out=out_d[t0:t0 + 128,
                                          db * 512:(db + 1) * 512],
                                in_=osb[:, db * 512:(db + 1) * 512],
                            )
                        elif osb_cur[key][1] == DOUT // 512:
                            nc.sync.dma_start(out=out_d[t0:t0 + 128, :],
                                              in_=osb[:])

                    def attn_head(qb, h, ctx_sb):
                        g = h // GS
                        pair = h // 2
                        rb = 64 * (h % 2)
                        even = (h % 2 == 0)
                        kc_lo = qb * (QB // 128)
                        nkc = (qb + 1) * (QB // 128)
                        kc_max = nkc - 1
                        qs0 = qb * QB
                        pctx = ctxps.tile([128, QB], f32, tag="ctx",
                                          name=f"pctx{qb}_{h}")
                        for kcp in range(0, nkc, 2):
                            pscr = scps.tile([128, 2 * QB], f32, tag="sc",
                                             name=f"sc{qb}_{h}_{kcp}")
                            for j in range(2):
                                kc = kcp + j
                                m = kc - kc_lo
                                a0 = m * 128 if m > 0 else 0
                                nc.tensor.matmul(
                                    pscr[:, j * QB + a0:(j + 1) * QB],
                                    kslice(g, rb, kc),
                                    QT[rb:rb + HDc, pair, qs0 + a0:qs0 + QB],
                                    start=True, stop=True,
                                )
                            m0 = kcp - kc_lo
                            c0 = m0 * 128 if m0 > 0 else 0
                            esb = expp.tile([128, 2 * QB], bf16, tag="exp")
                            if m0 == 2:
                                # skip the 384 masked-garbage columns in the
                                # second diagonal pair: two exact exps are
                                # cheaper than one spanning the hole
                                nc.scalar.activation(
                                    esb[:, 256:QB], pscr[:, 256:QB],
                                    mybir.ActivationFunctionType.Exp,
                                    scale=scale,
                                )
                                nc.scalar.activation(
                                    esb[:, QB + 384:2 * QB],
                                    pscr[:, QB + 384:2 * QB],
                                    mybir.ActivationFunctionType.Exp,
                                    scale=scale,
                                )
                            else:
                                nc.scalar.activation(
                                    esb[:, c0:2 * QB], pscr[:, c0:2 * QB],
                                    mybir.ActivationFunctionType.Exp,
                                    scale=scale,
                                )
                            for j in range(2):
                                kc = kcp + j
                                m = kc - kc_lo
                                if m >= 0:
                                    col = j * QB + m * 128
                                    nc.vector.tensor_mul(
                                        esb[:, col:col + 128],
                                        esb[:, col:col + 128], dmask[:])
                                a0 = m * 128 if m > 0 else 0
                                nc.tensor.matmul(
                                    pctx[0:HDc + 1, a0:QB],
                                    Vplus[g][:, kc, :],
                                    esb[:, j * QB + a0:(j + 1) * QB],
                                    start=(kc == 0), stop=(kc == kc_max),
                                )
                        den = densp.tile([1, QB], f32, tag="den")
                        nc.vector.reciprocal(den[:], pctx[HDc:HDc + 1, :])
                        # stage unnormalized ctx to SBUF immediately: frees
                        # the PSUM slot ~1.2us earlier; normalize from SBUF
                        cstg = densp.tile([64, QB], f32, tag="cstg")
                        nc.vector.tensor_copy(cstg[:], pctx[0:HDc, :])
                        bcst = densp.tile([64, QB], f32, tag="bcst")
                        nc.gpsimd.partition_broadcast(bcst[:], den[:],
                                                      channels=64)
                        if even:
                            nc.vector.tensor_mul(
                                ctx_sb[0:HDc, pair, :], cstg[:], bcst[:])
                        else:
                            ctmp = densp.tile([64, QB], bf16, tag="ctmp")
                            nc.vector.tensor_mul(ctmp[:], cstg[:], bcst[:])
                            nc.sync.dma_start(
                                out=ctx_sb[HDc:128, pair, :], in_=ctmp[:])

                    # V^T units assigned as PE fill: qb0 h0 gets all of tb0
                    # (needed by qb0 itself), then 2 per head
                    vt_fill = {(0, 0): [0, 1, 2, 3],
                               (0, 1): [4, 5], (0, 2): [6, 7],
                               (0, 3): [8, 9], (0, 4): [10, 11],
                               (0, 5): [12, 13], (0, 6): [14, 15]}

                    # outproj fill: (qb, h) -> list of (src_qb, block)
                    op_fill = {}
                    for h in range(HL):
                        op_fill[(1, h)] = [(0, 2 * h), (0, 2 * h + 1)]
                        op_fill[(2, h)] = [(1, 2 * h), (1, 2 * h + 1)]
                        op_fill[(3, h)] = ([(2, 2 * h), (2, 2 * h + 1)]
                                           if h < 5 else [(2, 10 + (h - 5))])

                    csb = {}
                    for qb in range(NQB):
                        csb[qb] = ctxp.tile([128, CC, QB], bf16,
                                            tag="ctx_sb", name=f"ctxsb{qb}")
                        for i, h in enumerate((1, 0, 3, 2, 5, 4, 7, 6)):
                            for sc_i in vt_fill.get((qb, i), []):
                                vt_unit(sc_i)
                            attn_head(qb, h, csb[qb])
                            for src, blk in op_fill.get((qb, i), []):
                                outproj_block(csb[src], src,
                                              blk // 4, blk % 4)
                    # remaining outproj(2) blocks cover the qb3 den drain
                    for blk in (13, 14, 15):
                        outproj_block(csb[2], 2, blk // 4, blk % 4)
                    # tail: out projection of the last query block,
                    # copies alternating between the idle ACT and DVE
                    for i in range(16):
                        tt, db = divmod(i, DOUT // 512)
                        outproj_block(csb[NQB - 1], NQB - 1, tt, db,
                                      tail=("act" if i % 2 else "dve"))

    nc.compile()
    return nc


def shard_inputs(x, cos, sin, Wq, Wk, Wv, Wo):
    """Build the 8 per-core input maps."""
    import ml_dtypes

    bf16 = ml_dtypes.bfloat16
    in_maps = []
    qw = H // TP * HD      # 512 q cols per quarter
    kw = G // TP * HD      # 128 k cols per quarter
    xT = [np.ascontiguousarray(x[b].T.astype(bf16)) for b in range(B)]
    ct = cos.T.astype(np.float32)
    st = sin.T.astype(np.float32)
    cosT = np.ascontiguousarray(np.concatenate([ct, ct], axis=0))
    sinT = np.ascontiguousarray(np.concatenate([st, st], axis=0))
    permT = np.zeros((128, 128), dtype=np.float32)
    for base in (0, 64):
        for i in range(32):
            permT[base + 32 + i, base + i] = -1.0
            permT[base + i, base + 32 + i] = 1.0
    permT = permT.astype(bf16)
    for c in range(N_CORES):
        b, qq = divmod(c, TP)
        wqkv = np.concatenate(
            [
                Wq[:, qq * qw:(qq + 1) * qw],
                Wk[:, qq * kw:(qq + 1) * kw],
                Wv[:, qq * kw:(qq + 1) * kw],
            ],
            axis=1,
        )
        in_maps.append(
            {
                "xT": xT[b],
                "wqkv": np.ascontiguousarray(wqkv.astype(bf16)),
                "wo": np.ascontiguousarray(
                    Wo[qq * qw:(qq + 1) * qw, :].astype(bf16)),
                "cosT": cosT,
                "sinT": sinT,
                "permT": permT,
            }
        )
    return in_maps


_NC_CACHE = {}


def _get_program():
    if "nc" not in _NC_CACHE:
        _NC_CACHE["nc"] = build_core_program(_cfg_full(), num_devices=N_CORES)
    return _NC_CACHE["nc"]


TRACE = False
LAST = {}


def kernel(x, attn_mask, cos, sin, Wq, Wk, Wv, Wo):
    from concourse.bass_utils import run_bass_kernel_spmd

    x = np.asarray(x)
    cos = np.asarray(cos)
    sin = np.asarray(sin)
    Wq, Wk, Wv, Wo = (np.asarray(a) for a in (Wq, Wk, Wv, Wo))

    nc = _get_program()
    in_maps = shard_inputs(x, cos, sin, Wq, Wk, Wv, Wo)
    r = run_bass_kernel_spmd(nc, in_maps, list(range(N_CORES)), trace=TRACE)
    LAST["results"] = r
    res = r.results

    y = np.zeros((B, S, D), dtype=np.float32)
    for c in range(N_CORES):
        y[c // TP] += res[c]["out"].astype(np.float32)
    return y
